# revision 11
# baseline (speedup 1.0000x reference)
"""ALiBi causal attention (B=2, S=2048, D=1024, H=16) on 8 TRN2 NeuronCores.

Strategy: tensor-parallel over heads. Core c owns heads (c, 15-c) — one
steep-slope (short ALiBi window) head and one shallow-slope head. Each core:
  - projects x -> Q^T, K^T (transposed layout, fp32r matmuls) and V (natural
    layout via PE transpose, bf16) for its two heads,
  - runs windowed causal flash-attention per head: scores S^T[k,q] computed
    with the ALiBi bias folded into the matmul via 3 exact "augmented" rows
    (values chosen exactly representable in bf16 so reduced-precision PE
    input rounding cannot corrupt the bias), single-pass exp (no running max
    needed: bias <= 0 keeps exponents bounded), P^T @ V accumulated in PSUM
    with a ones-column producing the softmax denominator for free,
  - applies its slice of W_O producing a partial [B*S, D] output.
Host sums the 8 partial outputs (the W_O all-reduce) and reshapes.

ALiBi sparsity: slopes decay 2^-0.5 .. 2^-8; for local head slot 0 (heads
0..7) k-tiles further than h7's window contribute < e^-25 and are skipped
structurally (same graph on all cores; steeper heads' extra tiles underflow
to exactly 0, which is correct).
"""

import math
import sys

sys.path.insert(0, "/opt/trn_rl_repo")

import ml_dtypes
import numpy as np

import concourse.bass as bass
import concourse.tile as tile
from concourse import bacc, mybir
from concourse.bass_utils import run_bass_kernel_spmd

B, S, D, H, HD = 2, 2048, 1024, 16, 64
P = 128          # k-tile / partition size
QC = 512         # q-chunk size (one PSUM bank of fp32)
NKT = S // P     # 16 k-tiles
NQC = S // QC    # 4 q-chunks
NCORES = 8
T_WIN = 25.0     # ln cutoff: drop k-tiles with alibi penalty > T_WIN
VW = HD + 1      # V columns per k-tile (64 V + 1 ones)

F32 = mybir.dt.float32
F32R = mybir.dt.float32r
BF16 = mybir.dt.bfloat16

# slopes m_h = 2^-(h+1)/2, snapped to bf16 so every aug-row value is exactly
# representable regardless of the PE's fp32r input rounding mode.
SLOPES = np.float32(ml_dtypes.bfloat16(2.0 ** (-(np.arange(H, dtype=np.float64) + 1) / 2.0)))


def _tiles_for_chunk(i: int, lh: int):
    """k-tiles kept for q-chunk i of local head slot lh (same on all cores).

    Slot 0 holds heads 0..7: the widest window among them is head 7
    (slope 2^-4). Slot 1 holds heads 8..15: full causal.
    """
    if lh == 0:
        L = T_WIN / float(SLOPES[7])
        t_lo = max(0, math.ceil((QC * i - L - (P - 1)) / P))
    else:
        t_lo = 0
    return list(range(t_lo, 4 * i + 4))


def build_graph() -> bass.Bass:
    nc = bacc.Bacc("TRN2", target_bir_lowering=False, debug=False)

    xT = nc.dram_tensor("xT", (B, D, S), BF16, kind="ExternalInput").ap()
    wq = nc.dram_tensor("wq", (D, P), BF16, kind="ExternalInput").ap()
    wk = nc.dram_tensor("wk", (D, P), BF16, kind="ExternalInput").ap()
    wv = nc.dram_tensor("wv", (D, P), BF16, kind="ExternalInput").ap()
    wo = nc.dram_tensor("wo", (P, D), BF16, kind="ExternalInput").ap()
    kaug = nc.dram_tensor("kaug", (2, 3, S), BF16, kind="ExternalInput").ap()
    qaug = nc.dram_tensor("qaug", (2, 3, S), BF16, kind="ExternalInput").ap()
    ebias = nc.dram_tensor("ebias", (2, 16), F32, kind="ExternalInput").ap()
    maskmin = nc.dram_tensor("maskmin", (4, P, QC), BF16, kind="ExternalInput").ap()
    ident = nc.dram_tensor("ident", (P, P), BF16, kind="ExternalInput").ap()
    out = nc.dram_tensor("out", (B * S, D), F32, kind="ExternalOutput").ap()
    recd = nc.dram_tensor("recd", (B, NQC, 2, QC), F32, kind="Internal").ap()

    with tile.TileContext(nc) as tc:
        with (
            tc.tile_pool(name="sb", bufs=1) as sb,
            tc.tile_pool(name="ps", bufs=1, space="PSUM") as ps,
        ):
            # ---- persistent SBUF ----
            wq_sb = sb.tile([P, 8, P], BF16, tag="wq")
            wk_sb = sb.tile([P, 8, P], BF16, tag="wk")
            wv_sb = sb.tile([P, 8, P], BF16, tag="wv")
            nc.gpsimd.dma_start(out=wq_sb, in_=wq.rearrange("(a p) m -> p a m", p=P))
            nc.gpsimd.dma_start(out=wk_sb, in_=wk.rearrange("(a p) m -> p a m", p=P))
            nc.gpsimd.dma_start(out=wv_sb, in_=wv.rearrange("(a p) m -> p a m", p=P))
            wo_sb = sb.tile([P, D], BF16, tag="wo")
            nc.gpsimd.dma_start(out=wo_sb, in_=wo)
            mm_sb = sb.tile([P, 4, QC], BF16, tag="mask")
            nc.gpsimd.dma_start(out=mm_sb, in_=maskmin.rearrange("j p q -> p j q"))
            id_sb = sb.tile([P, P], BF16, tag="ident")
            nc.gpsimd.dma_start(out=id_sb, in_=ident)
            eb_sb = sb.tile([P, 2, 16], F32, tag="ebias")
            eb_bcast = bass.AP(
                tensor=ebias.tensor, offset=0, ap=[[0, P], [16, 2], [1, 16]]
            )
            nc.gpsimd.dma_start(out=eb_sb, in_=eb_bcast)

            for b in range(B):
                QT = [
                    sb.tile([67, S], BF16, tag=f"qt{lh}", bufs=2, name=f"QT{lh}_{b}")
                    for lh in range(2)
                ]
                KT = [
                    sb.tile([67, S], BF16, tag=f"kt{lh}", bufs=2, name=f"KT{lh}_{b}")
                    for lh in range(2)
                ]
                Vb = [
                    sb.tile([P, NKT * VW], BF16, tag=f"v{lh}", bufs=2, name=f"V{lh}_{b}")
                    for lh in range(2)
                ]
                OTs = sb.tile([P, S], BF16, tag="ots", bufs=2, name=f"OTs_{b}")
                # ones everywhere; V data columns overwritten below, leaving
                # the per-tile ones column that yields the softmax denominator
                nc.vector.memset(Vb[0], 1.0)
                nc.vector.memset(Vb[1], 1.0)

                # ---- Q/K/V projections ----
                for i in range(NQC):
                    qps = ps.tile([P, QC], F32, tag="qps", name=f"qps_{b}_{i}")
                    kps = ps.tile([P, QC], F32, tag="kps", name=f"kps_{b}_{i}")
                    vps = ps.tile([P, QC], F32, tag="vps", bufs=2, name=f"vps_{b}_{i}")
                    xt = sb.tile([P, 8, QC], BF16, tag="xt", bufs=3, name=f"xt_{b}_{i}")
                    nc.gpsimd.dma_start(
                        out=xt,
                        in_=xT[b, :, QC * i : QC * (i + 1)].rearrange(
                            "(a p) q -> p a q", p=P
                        ),
                    )
                    for k in range(8):
                        st, sp = (k == 0), (k == 7)
                        nc.tensor.matmul(
                            qps, wq_sb[:, k, :], xt[:, k, :],
                            start=st, stop=sp,
                        )
                        nc.tensor.matmul(
                            kps, wk_sb[:, k, :], xt[:, k, :],
                            start=st, stop=sp,
                        )
                        nc.tensor.matmul(
                            vps, wv_sb[:, k, :], xt[:, k, :],
                            start=st, stop=sp,
                        )
                    cs = slice(QC * i, QC * (i + 1))
                    # head slot 0 rows land on matching partitions: direct copy
                    nc.vector.tensor_copy(out=QT[0][0:64, cs], in_=qps[0:64, :])
                    nc.vector.tensor_copy(out=KT[0][0:64, cs], in_=kps[0:64, :])
                    # head slot 1 rows must shift partitions 64:128 -> 0:64:
                    # stage on ACT, move with an SBUF->SBUF DMA
                    qstage = sb.tile([P, QC], BF16, tag="qstage", bufs=2, name=f"qs_{b}_{i}")
                    kstage = sb.tile([P, QC], BF16, tag="kstage", bufs=2, name=f"ks_{b}_{i}")
                    nc.scalar.copy(out=qstage[64:128, :], in_=qps[64:128, :])
                    nc.scalar.copy(out=kstage[64:128, :], in_=kps[64:128, :])
                    nc.gpsimd.dma_start(out=QT[1][0:64, cs], in_=qstage[64:128, :])
                    nc.gpsimd.dma_start(out=KT[1][0:64, cs], in_=kstage[64:128, :])
                    # V: cast to bf16, PE-transpose each 128 block to natural
                    vt16 = sb.tile([P, QC], BF16, tag="vt16", bufs=2, name=f"vt_{b}_{i}")
                    nc.vector.tensor_copy(out=vt16, in_=vps)
                    for j in range(4):
                        sti = 4 * i + j
                        vn = ps.tile([P, P], BF16, tag="st", bufs=2, name=f"vn_{b}_{sti}")
                        nc.tensor.transpose(
                            out=vn, in_=vt16[:, P * j : P * (j + 1)], identity=id_sb
                        )
                        nc.vector.tensor_copy(
                            out=Vb[0][:, sti * VW : sti * VW + HD], in_=vn[:, 0:64]
                        )
                        nc.vector.tensor_copy(
                            out=Vb[1][:, sti * VW : sti * VW + HD], in_=vn[:, 64:128]
                        )

                # ALiBi augmented rows (exact-in-bf16 values)
                for lh in range(2):
                    nc.gpsimd.dma_start(out=QT[lh][64:67, :], in_=qaug[lh])
                    nc.gpsimd.dma_start(out=KT[lh][64:67, :], in_=kaug[lh])

                # ---- attention + output projection, chunk by chunk ----
                for i in range(NQC):
                    cs = slice(QC * i, QC * (i + 1))
                    for lh in range(2):
                        tiles = _tiles_for_chunk(i, lh)
                        otp = ps.tile(
                            [65, QC], F32, tag="ot", bufs=2, name=f"ot_{b}_{i}_{lh}"
                        )
                        for t in tiles:
                            stp = ps.tile(
                                [P, QC], F32, tag="st", bufs=2, name=f"st_{b}_{i}_{lh}_{t}"
                            )
                            nc.tensor.matmul(
                                stp,
                                KT[lh][0:67, P * t : P * (t + 1)],
                                QT[lh][0:67, cs],
                                start=True, stop=True,
                            )
                            pt = sb.tile(
                                [P, QC], BF16, tag="pt", bufs=4, name=f"pt_{b}_{i}_{lh}_{t}"
                            )
                            idx = 4 * i - t + 3
                            nc.scalar.activation(
                                out=pt, in_=stp,
                                func=mybir.ActivationFunctionType.Exp,
                                bias=eb_sb[:, lh, idx : idx + 1], scale=1.0,
                            )
                            j = t - 4 * i
                            if j >= 0:  # diagonal tile: zero k>q via min-mask
                                nc.vector.tensor_tensor(
                                    out=pt, in0=pt, in1=mm_sb[:, j, :],
                                    op=mybir.AluOpType.min,
                                )
                            nc.tensor.matmul(
                                otp[0:65, :],
                                Vb[lh][:, t * VW : (t + 1) * VW],
                                pt,
                                start=(t == tiles[0]), stop=(t == tiles[-1]),
                            )
                        # softmax denominator -> reciprocal -> broadcast
                        rec = sb.tile([65, QC], F32, tag="rec", bufs=2, name=f"rec_{b}_{i}_{lh}")
                        nc.vector.reciprocal(out=rec[64:65, :], in_=otp[64:65, :])
                        nc.gpsimd.dma_start(out=recd[b, i, lh], in_=rec[64:65, :])
                        bc = sb.tile([64, QC], F32, tag="bc", bufs=2, name=f"bc_{b}_{i}_{lh}")
                        rsl = recd[b, i, lh]
                        bc_src = bass.AP(
                            tensor=rsl.tensor, offset=rsl.offset, ap=[[0, 64]] + list(rsl.ap)
                        )
                        nc.gpsimd.dma_start(out=bc, in_=bc_src)
                        if lh == 0:
                            nc.vector.tensor_tensor(
                                out=OTs[0:64, cs], in0=otp[0:64, :], in1=bc,
                                op=mybir.AluOpType.mult,
                            )
                        else:
                            ott = sb.tile([64, QC], BF16, tag="ott", bufs=2, name=f"ott_{b}_{i}")
                            nc.vector.tensor_tensor(
                                out=ott, in0=otp[0:64, :], in1=bc,
                                op=mybir.AluOpType.mult,
                            )
                            nc.gpsimd.dma_start(out=OTs[64:128, cs], in_=ott)

                    # output projection for this chunk's 4 s-tiles
                    for j in range(4):
                        sti = 4 * i + j
                        for n in range(2):
                            ops_t = ps.tile(
                                [P, QC], F32, tag="vps", bufs=2, name=f"op_{b}_{sti}_{n}"
                            )
                            nc.tensor.matmul(
                                ops_t,
                                OTs[:, P * sti : P * (sti + 1)],
                                wo_sb[:, QC * n : QC * (n + 1)],
                                start=True, stop=True,
                            )
                            ost = sb.tile(
                                [P, QC], F32, tag="ost", bufs=4, name=f"os_{b}_{sti}_{n}"
                            )
                            if n == 0:
                                nc.vector.tensor_copy(out=ost, in_=ops_t)
                            else:
                                nc.scalar.copy(out=ost, in_=ops_t)
                            nc.sync.dma_start(
                                out=out[
                                    b * S + P * sti : b * S + P * (sti + 1),
                                    QC * n : QC * (n + 1),
                                ],
                                in_=ost,
                            )
    nc.compile()
    return nc


_NC_CACHE = None


def _get_graph():
    global _NC_CACHE
    if _NC_CACHE is None:
        _NC_CACHE = build_graph()
    return _NC_CACHE


def _host_inputs(x, W_Q, W_K, W_V, W_O):
    """Per-core input maps."""
    x = np.asarray(x, dtype=np.float32)
    W_Q = np.asarray(W_Q, dtype=np.float32)
    W_K = np.asarray(W_K, dtype=np.float32)
    W_V = np.asarray(W_V, dtype=np.float32)
    W_O = np.asarray(W_O, dtype=np.float32)

    xT = ml_dtypes.bfloat16(np.ascontiguousarray(x.transpose(0, 2, 1)))  # [B, D, S]

    karr = np.arange(S, dtype=np.float64)
    kk = np.float32(karr % P)                  # 0..127, exact
    qh = np.float32((karr % QC) // 32)         # 0..15, exact
    ql = np.float32((karr % QC) % 32)          # 0..31, exact
    ones = np.ones(S, dtype=np.float32)

    mask = np.zeros((4, P, QC), dtype=np.float64)
    kki = np.arange(P)[:, None]
    qqi = np.arange(QC)[None, :]
    for j in range(4):
        mask[j] = np.where(kki + P * j <= qqi, 1e9, 0.0)
    maskmin = ml_dtypes.bfloat16(mask)
    ident = np.eye(P, dtype=ml_dtypes.bfloat16)

    in_maps = []
    for c in range(NCORES):
        heads = (c, H - 1 - c)
        rows = np.concatenate(
            [np.arange(64 * h, 64 * h + 64) for h in heads]
        )
        wq = ml_dtypes.bfloat16(np.ascontiguousarray((W_Q[rows, :] / 8.0).T))
        wk = ml_dtypes.bfloat16(np.ascontiguousarray(W_K[rows, :].T))
        wv = ml_dtypes.bfloat16(np.ascontiguousarray(W_V[rows, :].T))
        wo = ml_dtypes.bfloat16(np.ascontiguousarray(W_O[:, rows].T))

        kaug = np.zeros((2, 3, S), dtype=np.float32)
        qaug = np.zeros((2, 3, S), dtype=np.float32)
        eb = np.zeros((2, 16), dtype=np.float32)
        for lh, h in enumerate(heads):
            sl = float(SLOPES[h])
            kaug[lh, 0] = kk
            kaug[lh, 1] = np.float32(-32.0 * sl)
            kaug[lh, 2] = np.float32(-sl)
            qaug[lh, 0] = np.float32(sl) * ones
            qaug[lh, 1] = qh
            qaug[lh, 2] = ql
            eb[lh] = np.float32(-P * sl * (np.arange(16, dtype=np.float64) - 3.0))
        in_maps.append(
            {
                "xT": xT,
                "wq": wq,
                "wk": wk,
                "wv": wv,
                "wo": wo,
                "kaug": ml_dtypes.bfloat16(kaug),
                "qaug": ml_dtypes.bfloat16(qaug),
                "ebias": eb,
                "maskmin": maskmin,
                "ident": ident,
            }
        )
    return in_maps


LAST_RESULTS = None


def kernel(x, W_Q, W_K, W_V, W_O):
    global LAST_RESULTS
    nc = _get_graph()
    in_maps = _host_inputs(x, W_Q, W_K, W_V, W_O)
    res = run_bass_kernel_spmd(nc, in_maps, core_ids=list(range(NCORES)))
    LAST_RESULTS = res
    total = np.zeros((B * S, D), dtype=np.float32)
    for r in res.results:
        total += r["out"]
    return total.reshape(B, S, D)


if __name__ == "__main__":
    nc = build_graph()
    print("graph built ok")
